# revision 37
# baseline (speedup 1.0000x reference)
"""Trainium2 Bass kernel for nn_DARPDecoder (sparse_attention).

Strategy (pure data-parallel over batch, 8 cores x 128 batches):
  score[b,n] = emb[b,n,:].qk[b]/sqrt(D) - travel[b,n]*c ; tanh-clip, mask,
  log_softmax, with qk[b] = W_key^T q[b] (no [B,N,D] K intermediate).
Per core, TWO HBM streams consumed in a chunk-wise pipeline:
  - nat_f8 (natural [n,d] tiles, fp8-e4m3, 8.4MB, 8 chunks of 16 batches):
    graph/visited sums via "flipped" matmuls -- the nat tile is the
    STATIONARY, the moving operand a tiny [128,2] (0.25 | 0.25*vf) column
    pair accumulating [128d,2] per batch.  fp8 is plenty for the sums.
  - et_bf (transposed [d,n] tiles, bf16, 16.8MB, 16 half-chunks): per-batch
    score matmuls with zero-padded qk columns (batch b -> col 32b+(b%32))
    accumulate into a [128b, 512n] PSUM, 32-row band per batch group.
Streams are ordered nat-first so every group's sums -> 1/vcount descale
(exact transpose sandwich) -> q -> qk completes while et streams in; scores
then chase each et half-chunk; the epilogue runs per 32-row band as its band
completes.  All consts ship as ONE u8 blob DMA (bitcast views); ~30 DMAs
total on one queue keeps the shared HWDGE/SEQ path off the critical path.
Travel: T rows gathered by cur_h3 (indirect DMA, bf16, pre-scaled by c via
16 selection matmuls that replicate rows across 16-partition groups), then
gpsimd indirect_copy gathers T[cur_h3[b], h3[b,n]], and 16 "select -1 rows"
matmuls accumulate -travel straight into the score PSUM (a zero-stationary
matmul provides the start=True clear, making accumulation order-free).
Epilogue per band: 10*tanh(x/10) = 10 - 20/(exp(x/5)+1) so tanh/exp/ln all
live in ONE activation-table set (a dummy Exp after each Ln re-pins it);
post-tanh scores are in [-10,10] so log-sum-exp needs no max shift.
"""

import functools
import math

import numpy as np
import ml_dtypes

import concourse.bass as bass
import concourse.mybir as mybir
import concourse.tile as tile
from concourse import bacc
from concourse.bass_utils import run_bass_kernel_spmd

BF16 = mybir.dt.bfloat16
F8 = mybir.dt.float8e4
F32 = mybir.dt.float32
I32 = mybir.dt.int32
U16 = mybir.dt.uint16
U8 = mybir.dt.uint8
Alu = mybir.AluOpType
AF = mybir.ActivationFunctionType
AX = mybir.AxisListType

B, N, D, NCORES = 1024, 512, 128, 8
BC = B // NCORES   # 128 batches/core
NG, GB = 8, 16     # 8 groups x 16 batches
MAX_TIME = 1440.0
TANH_CLIP = 10.0
C_TRAVEL = 1.0 / MAX_TIME / math.sqrt(2.0)
INV_SQRT_D = 1.0 / math.sqrt(D)
NBF = np.dtype(ml_dtypes.bfloat16)
NF8 = np.dtype(ml_dtypes.float8_e4m3)
BLOB_BYTES = 13096
DEBUG_TAPS = False


def _emit(nc, tc, T):
    ap = {k: v.ap() for k, v in T.items()}

    with (
        tc.tile_pool(name="cp", bufs=1) as cp,
        tc.tile_pool(name="wk", bufs=2) as wk,
        tc.tile_pool(name="stn", bufs=6) as stn,
        tc.tile_pool(name="ste", bufs=10) as ste,
        tc.tile_pool(name="ps_sc", bufs=1, space="PSUM") as ps_sc,
        tc.tile_pool(name="ps_rep", bufs=2, space="PSUM") as ps_rep,
        tc.tile_pool(name="ps_sm", bufs=2, space="PSUM") as ps_sm,
        tc.tile_pool(name="ps_q", bufs=2, space="PSUM") as ps_q,
    ):
        # ---- DMA issue: one consts blob, then the two streams, all on SP
        #      (one HWDGE slot per big transfer; never issue-rate bound) ----
        blob = cp.tile([128, BLOB_BYTES], U8, name="blob")
        nc.sync.dma_start(out=blob[:], in_=ap["blob"])
        sc8 = blob[:, 0:32].bitcast(F32)
        iota = blob[:, 32:36].bitcast(F32)
        bst = blob[:, 36:40].bitcast(F32)
        vfc = blob[:, 40:1064].bitcast(F8)
        visam = blob[:, 1064:2088]
        wcat = blob[:, 2088:3368].bitcast(BF16)
        ws = blob[:3, 3368:3624].bitcast(BF16)
        idn = blob[:, 3624:3880].bitcast(BF16)
        h3w = blob[:, 3880:4904].bitcast(U16)
        sselc = blob[:, 4904:9000].bitcast(BF16)
        dkc = blob[:, 9000:13096].bitcast(BF16)

        nat = [None] * NG
        for g in range(NG):
            nat[g] = stn.tile([128, GB * 4 * D], F8, tag="nat", name=f"nat{g}")
            nc.sync.dma_start(out=nat[g][:], in_=ap["nat_f8"][g])
        et = [None] * 16
        for h in range(16):
            et[h] = ste.tile([128, 8 * N], BF16, tag="et", name=f"et{h}")
            nc.sync.dma_start(out=et[h][:], in_=ap["et_bf"][h])

        # zero stationary for the pssc-clearing matmul
        zc = cp.tile([128, 128], BF16, name="zc")
        nc.vector.memset(zc[:], 0.0)

        # warm the ACT table set (copy/exp/ln all live in one set)
        actw = cp.tile([1, 1], F32, name="actw")
        nc.vector.memset(actw[:], 0.0)
        nc.scalar.activation(out=actw[:], in_=actw[:], func=AF.Exp)

        # ---- qkp (zero-padded qk columns), memset early: zero deps ----
        qkp = cp.tile([128, 32 * BC], BF16, name="qkp")
        nc.vector.memset(qkp[:], 0.0)

        # ---- scalar state / first-node bookkeeping (DVE, f32 exact) ----
        t1 = cp.tile([BC, 1], F32, name="t1")
        nc.vector.tensor_single_scalar(out=t1[:], in_=sc8[:, 5:6], scalar=0.0,
                                       op=Alu.is_equal)
        t2 = cp.tile([BC, 1], F32, name="t2")
        nc.vector.tensor_single_scalar(out=t2[:], in_=sc8[:, 4:5], scalar=0.0,
                                       op=Alu.not_equal)
        ld = cp.tile([BC, 1], F32, name="ld")
        nc.vector.tensor_mul(out=ld[:], in0=t1[:], in1=t2[:])
        dd = cp.tile([BC, 1], F32, name="dd")
        nc.vector.tensor_sub(out=dd[:], in0=sc8[:, 4:5], in1=sc8[:, 6:7])
        nc.vector.tensor_mul(out=dd[:], in0=ld[:], in1=dd[:])
        fnf = cp.tile([BC, 1], F32, name="fnf")
        nc.vector.tensor_add(out=fnf[:], in0=sc8[:, 6:7], in1=dd[:])
        nc.vector.tensor_mul(out=fnf[:], in0=fnf[:], in1=t2[:])

        gcf = cp.tile([BC, 1], F32, name="gcf")
        nc.vector.tensor_add(out=gcf[:], in0=iota, in1=sc8[:, 4:5])
        gcur = cp.tile([BC, 1], I32, name="gcur")
        nc.vector.tensor_copy(out=gcur[:], in_=gcf[:])
        gff = cp.tile([BC, 1], F32, name="gff")
        nc.vector.tensor_add(out=gff[:], in0=iota, in1=fnf[:])
        gfn = cp.tile([BC, 1], I32, name="gfn")
        nc.vector.tensor_copy(out=gfn[:], in_=gff[:])

        # visited count -> vcr4 = 4/max(vc,1) (bf16 col for ident matmul rhs)
        visf = cp.tile([BC, N], F32, name="visf")
        nc.vector.tensor_copy(out=visf[:], in_=visam[:, :N])
        vc = cp.tile([BC, 1], F32, name="vc")
        nc.vector.tensor_reduce(out=vc[:], in_=visf[:], axis=AX.X, op=Alu.add)
        nc.vector.tensor_scalar_max(out=vc[:], in0=vc[:], scalar1=1.0)
        nc.vector.tensor_scalar_mul(out=vc[:], in0=vc[:], scalar1=0.25)
        vcr4 = cp.tile([BC, 1], F32, name="vcr4")
        nc.vector.reciprocal(out=vcr4[:], in_=vc[:])
        vcr4b = cp.tile([BC, 1], BF16, name="vcr4b")
        nc.vector.tensor_copy(out=vcr4b[:], in_=vcr4[:])

        # action mask precompute
        amf = cp.tile([BC, N], F32, name="amf")
        nc.vector.tensor_copy(out=amf[:], in_=visam[:, N:])
        mA = cp.tile([BC, N], F32, name="mA")
        nc.vector.tensor_scalar(out=mA[:], in0=amf[:], scalar1=1.0, scalar2=1e8,
                                op0=Alu.subtract, op1=Alu.mult)
        m10 = cp.tile([BC, N], F32, name="m10")
        nc.vector.tensor_scalar_mul(out=m10[:], in0=amf[:], scalar1=TANH_CLIP)
        nc.vector.tensor_add(out=mA[:], in0=mA[:], in1=m10[:])
        mB = cp.tile([BC, N], F32, name="mB")
        nc.vector.tensor_scalar_mul(out=mB[:], in0=amf[:], scalar1=2.0 * TANH_CLIP)

        # state feats [BC,3] -> sft [3,BC]
        sfb = cp.tile([BC, 3], BF16, name="sfb")
        nc.vector.tensor_sub(out=sfb[:, 0:1], in0=sc8[:, 2:3], in1=sc8[:, 1:2])
        nc.vector.tensor_scalar_mul(out=sfb[:, 1:2], in0=sc8[:, 0:1],
                                    scalar1=1.0 / MAX_TIME)
        nc.vector.tensor_scalar_mul(out=sfb[:, 2:3], in0=sc8[:, 3:4],
                                    scalar1=1.0 / (2.0 * N))
        psf = ps_q.tile([128, 128], BF16, tag="sm")
        nc.tensor.transpose(out=psf[:3, :], in_=sfb[:], identity=idn)
        sft = cp.tile([3, BC], BF16, name="sft")
        nc.vector.tensor_copy(out=sft[:], in_=psf[:3, :BC])

        # wg scaled by 1/128 (sums use 0.25 weights; graph mean needs /512)
        wgs = cp.tile([D, D], BF16, name="wgs")
        nc.vector.tensor_scalar_mul(out=wgs[:], in0=wcat[:, 2 * D:3 * D],
                                    scalar1=1.0 / 128.0)

        # ---- gathers (Pool/SWDGE queue) ----
        hc_rows = cp.tile([BC, D], BF16, name="hc_rows")
        nc.gpsimd.indirect_dma_start(
            out=hc_rows, out_offset=None, in_=ap["emb_flat"],
            in_offset=bass.IndirectOffsetOnAxis(ap=gcur[:, :1], axis=0))
        hf_rows = cp.tile([BC, D], BF16, name="hf_rows")
        nc.gpsimd.indirect_dma_start(
            out=hf_rows, out_offset=None, in_=ap["emb_flat"],
            in_offset=bass.IndirectOffsetOnAxis(ap=gfn[:, :1], axis=0))
        ch3 = cp.tile([BC, 1], I32, name="ch3")
        nc.gpsimd.indirect_dma_start(
            out=ch3[:], out_offset=None, in_=ap["h3_flat"],
            in_offset=bass.IndirectOffsetOnAxis(ap=gcur[:, :1], axis=0))
        rrow = cp.tile([BC, N], BF16, name="rrow")
        nc.gpsimd.indirect_dma_start(
            out=rrow[:], out_offset=None, in_=ap["ttm_bf"],
            in_offset=bass.IndirectOffsetOnAxis(ap=ch3[:, :1], axis=0))

        # ---- h_cur / h_first transposes -> [128d, BC] bf16 ----
        hct = cp.tile([D, BC], BF16, name="hct")
        pt1 = ps_q.tile([128, 128], BF16, tag="sm")
        nc.tensor.transpose(out=pt1[:], in_=hc_rows, identity=idn)
        nc.vector.tensor_copy(out=hct[:], in_=pt1[:])
        hft = cp.tile([D, BC], BF16, name="hft")
        pt2 = ps_q.tile([128, 128], BF16, tag="sm")
        nc.tensor.transpose(out=pt2[:], in_=hf_rows, identity=idn)
        nc.vector.tensor_copy(out=hft[:], in_=pt2[:])

        gk_all = cp.tile([128, 16 * N], BF16, name="gk_all")
        pssc = ps_sc.tile([128, N], F32, tag="score")
        # clear pssc once; every later matmul (travel + scores) accumulates
        nc.tensor.matmul(out=pssc[:], lhsT=zc[:], rhs=qkp[:, :N], start=True,
                         stop=False, skip_group_check=True)

        # ---- loop A: per-group sums -> qk (chases the nat stream); travel
        #      replication/gather interleaved 2 calls per group ----
        for g in range(NG):
            # sums: flipped matmuls, nat tile slices as stationaries
            psS = ps_sm.tile([128, 2 * GB], F32, tag="sums")
            for j in range(GB):
                for t in range(4):
                    nc.tensor.matmul(
                        out=psS[:, 2 * j:2 * j + 2],
                        lhsT=nat[g][:, (j * 4 + t) * D:(j * 4 + t + 1) * D],
                        rhs=vfc[:, 128 * g + 32 * t + 2 * j:
                                 128 * g + 32 * t + 2 * j + 2],
                        start=(t == 0), stop=(t == 3), skip_group_check=True)

            # graph cols (even) / raw visited cols (odd) -> SBUF bf16
            gt_g = wk.tile([D, GB], BF16, tag="gt")
            nc.vector.tensor_copy(
                out=gt_g[:], in_=psS[:].rearrange("p (s c) -> p s c", c=2)[:, :, 0])
            vr_g = wk.tile([D, GB], BF16, tag="vr")
            nc.vector.tensor_copy(
                out=vr_g[:], in_=psS[:].rearrange("p (s c) -> p s c", c=2)[:, :, 1])

            # 1/vcount descale sandwich: transpose, per-partition scale, back
            vcg = ps_q.tile([GB, 1], F32, tag="sm")
            nc.tensor.matmul(out=vcg[:], lhsT=idn[:, GB * g:GB * (g + 1)],
                             rhs=vcr4b[:], start=True, stop=True)
            vcgs = wk.tile([GB, 1], F32, tag="vcgs")
            nc.vector.tensor_copy(out=vcgs[:], in_=vcg[:])
            pvt = ps_q.tile([GB, D], BF16, tag="sm")
            nc.tensor.transpose(out=pvt[:], in_=vr_g[:], identity=idn)
            vts = wk.tile([GB, D], BF16, tag="vts")
            nc.vector.tensor_scalar(out=vts[:], in0=pvt[:], scalar1=vcgs[:, :1],
                                    scalar2=None, op0=Alu.mult)
            pvb = ps_q.tile([D, GB], F32, tag="sm")
            nc.tensor.matmul(out=pvb[:], lhsT=vts[:], rhs=idn[:GB, :GB],
                             start=True, stop=True)
            vt_g = wk.tile([D, GB], BF16, tag="vt")
            nc.vector.tensor_copy(out=vt_g[:], in_=pvb[:])

            # q = W_last^T hc + W_first^T hf + Wg' G + Wv V + W_state^T sf (+b)
            psq = ps_q.tile([D, GB], F32, tag="sm")
            nc.tensor.matmul(out=psq[:], lhsT=wcat[:, 0:D],
                             rhs=hct[:, GB * g:GB * (g + 1)], start=True, stop=True)
            nc.tensor.matmul(out=psq[:], lhsT=wcat[:, D:2 * D],
                             rhs=hft[:, GB * g:GB * (g + 1)], start=False,
                             stop=True, skip_group_check=True)
            nc.tensor.matmul(out=psq[:], lhsT=wgs[:], rhs=gt_g[:], start=False,
                             stop=True, skip_group_check=True)
            nc.tensor.matmul(out=psq[:], lhsT=wcat[:, 3 * D:4 * D], rhs=vt_g[:],
                             start=False, stop=True, skip_group_check=True)
            nc.tensor.matmul(out=psq[:], lhsT=ws,
                             rhs=sft[:, GB * g:GB * (g + 1)], start=False,
                             stop=True, skip_group_check=True)
            qt_g = wk.tile([D, GB], BF16, tag="qt")
            nc.vector.tensor_scalar(out=qt_g[:], in0=psq[:], scalar1=bst[:, :1],
                                    scalar2=None, op0=Alu.add)

            # qk = W_key^T q / sqrt(D)
            psk = ps_q.tile([D, GB], F32, tag="sm")
            nc.tensor.matmul(out=psk[:], lhsT=wcat[:, 4 * D:5 * D], rhs=qt_g[:],
                             start=True, stop=True)
            qk_g = wk.tile([D, GB], BF16, tag="qkg")
            nc.vector.tensor_scalar_mul(out=qk_g[:], in0=psk[:],
                                        scalar1=INV_SQRT_D)

            # scatter into qkp: batch b=16g+j -> col 32b + (b%32)
            base = 512 * g + 16 * (g % 2)
            nc.vector.tensor_copy(out=qkp[:, base:base + 33 * (GB - 1) + 1:33],
                                  in_=qk_g[:])

            # travel replication + gpsimd gather, 2 calls per group
            for k in (2 * g, 2 * g + 1):
                prep = ps_rep.tile([128, N], F32, tag="rep")
                nc.tensor.matmul(out=prep[:], lhsT=sselc[:, 128 * k:128 * (k + 1)],
                                 rhs=rrow[:], start=True, stop=True)
                sck = wk.tile([128, N], BF16, tag="sck")
                nc.scalar.activation(out=sck[:], in_=prep[:], func=AF.Copy)
                nc.gpsimd.indirect_copy(out=gk_all[:, N * k:N * (k + 1)],
                                        data=sck[:],
                                        idxs=h3w[:, 32 * k:32 * (k + 1)],
                                        i_know_ap_gather_is_preferred=True)

        # ---- travel select-accumulate straight into the score PSUM ----
        for k in range(16):
            nc.tensor.matmul(out=pssc[:], lhsT=dkc[:, 128 * k:128 * (k + 1)],
                             rhs=gk_all[:, N * k:N * (k + 1)],
                             start=False, stop=False, skip_group_check=True)

        # ---- loop B: per-half-group scores (chases the et stream), with the
        #      epilogue + output DMA emitted per 32-row band as it completes ----
        th = cp.tile([BC, N], F32, name="th")
        msk = cp.tile([BC, N], F32, name="msk")
        ex = cp.tile([BC, N], F32, name="ex")
        fin = cp.tile([BC, N], F32, name="fin")
        se = cp.tile([BC, 2], F32, name="se")
        set_ = cp.tile([BC, 1], F32, name="set_")
        lse = cp.tile([BC, 1], F32, name="lse")
        for h in range(16):
            J = h // 4
            for j in range(8):
                b = 8 * h + j
                nc.tensor.matmul(
                    out=pssc[32 * J:32 * J + 32, :],
                    lhsT=qkp[:, 32 * b:32 * b + 32],
                    rhs=et[h][:, N * j:N * (j + 1)],
                    start=False, stop=(b % 32 == 31),
                    tile_position=(0, 32 * J), skip_group_check=True)
            if h % 4 == 3:
                sl = slice(32 * J, 32 * J + 32)
                # 10*tanh(x/10) = 10 - 20/(exp(x/5)+1): stays in the exp/ln
                # act-table set (no per-band table reloads).  Post-tanh scores
                # are clipped to [-10,10], so log-sum-exp needs no max shift.
                # The last band is column-split so the serial op chain
                # pipelines across ACT and DVE in the tail.
                halves = (slice(0, N),) if J < 3 else (slice(0, N // 2),
                                                       slice(N // 2, N))
                for ci, cs in enumerate(halves):
                    nc.scalar.activation(out=th[sl, cs], in_=pssc[sl, cs],
                                         func=AF.Exp, scale=2.0 / TANH_CLIP)
                    nc.vector.tensor_scalar_add(out=th[sl, cs], in0=th[sl, cs],
                                                scalar1=1.0)
                    nc.vector.reciprocal(out=th[sl, cs], in_=th[sl, cs])
                    nc.vector.tensor_mul(out=th[sl, cs], in0=th[sl, cs],
                                         in1=mB[sl, cs])
                    nc.vector.tensor_sub(out=msk[sl, cs], in0=mA[sl, cs],
                                         in1=th[sl, cs])
                    nc.scalar.activation(out=ex[sl, cs], in_=msk[sl, cs],
                                         func=AF.Exp, scale=1.0,
                                         accum_out=se[sl, ci:ci + 1])
                if J == 3:
                    nc.vector.tensor_add(out=set_[sl], in0=se[sl, 0:1],
                                         in1=se[sl, 1:2])
                    nc.scalar.activation(out=lse[sl], in_=set_[sl], func=AF.Ln)
                else:
                    nc.scalar.activation(out=lse[sl], in_=se[sl, 0:1],
                                         func=AF.Ln)
                nc.vector.tensor_scalar(out=fin[sl], in0=msk[sl],
                                        scalar1=lse[sl.start:sl.stop, :1],
                                        scalar2=None, op0=Alu.subtract)
                nc.sync.dma_start(out=ap["out"][sl], in_=fin[sl])
                if J < 3:
                    # re-pin the exp table so the next band's Exp needs no
                    # act-table reload (Ln lives in a different set)
                    nc.scalar.activation(out=actw[:], in_=actw[:], func=AF.Exp)



def build_program():
    nc = bacc.Bacc("TRN2", target_bir_lowering=False, debug=False)
    dt = nc.dram_tensor
    T = {}

    def din(name, shape, dtype):
        T[name] = dt(name, shape, dtype, kind="ExternalInput")

    din("nat_f8", [NG, 128, GB * 4 * D], F8)
    din("et_bf", [16, 128, 8 * N], BF16)
    din("emb_flat", [BC * N, D], BF16)
    din("h3_flat", [BC * N, 1], I32)
    din("ttm_bf", [N, N], BF16)
    din("blob", [128, BLOB_BYTES], U8)
    T["out"] = dt("out", [BC, N], F32, kind="ExternalOutput")
    if DEBUG_TAPS:
        for nm, shp in [("d_hct", [D, BC]), ("d_hft", [D, BC]),
                        ("d_qkp", [128, 32 * BC]), ("d_trav", [BC, N]),
                        ("d_score", [BC, N])]:
            T[nm] = dt(nm, shp, F32, kind="ExternalOutput")

    with tile.TileContext(nc) as tc:
        _emit(nc, tc, T)
    nc.compile()
    return nc


@functools.cache
def _cached_program():
    return build_program()


@functools.cache
def _consts():
    c = {}
    c["ident"] = np.eye(128, dtype=NBF)
    s = np.zeros((16, 128, 128), dtype=np.float32)
    dk = np.zeros((16, 128, 128), dtype=np.float32)
    pidx = np.arange(128)
    for k in range(16):
        s[k, (pidx // 16) * 16 + k, pidx] = C_TRAVEL
        rows = pidx[pidx % 16 == k]
        dk[k, rows, rows] = -1.0
    c["sselc"] = np.ascontiguousarray(s.transpose(1, 0, 2)).reshape(128, 2048).astype(NBF)
    c["dkc"] = np.ascontiguousarray(dk.transpose(1, 0, 2)).reshape(128, 2048).astype(NBF)
    c["iota"] = (np.arange(BC, dtype=np.float32) * N)[:, None]
    return c


def make_in_map(inputs, core, consts=None):
    """Host-side shard + relayout for one core (pure layout/dtype work)."""
    sl = slice(BC * core, BC * (core + 1))
    emb = np.asarray(inputs["node_emb"][sl], dtype=np.float32)
    embb = emb.astype(NBF)          # [128, 512, 128]
    embf8 = emb.astype(NF8)
    m = {}
    m["nat_f8"] = np.ascontiguousarray(
        embf8.reshape(NG, GB, 4, 128, D).transpose(0, 3, 1, 2, 4)
    ).reshape(NG, 128, GB * 4 * D)
    m["et_bf"] = np.ascontiguousarray(
        embb.transpose(0, 2, 1).reshape(16, 8, D, N).transpose(0, 2, 1, 3)
    ).reshape(16, 128, 8 * N)
    m["emb_flat"] = embb.reshape(BC * N, D)
    h3 = np.asarray(inputs["h3_indices"][sl]).astype(np.int32)   # [128, 512]
    m["h3_flat"] = h3.reshape(BC * N, 1)
    h3w = np.ascontiguousarray(
        h3.reshape(8, 16, 32, 16).transpose(1, 0, 3, 2).reshape(16, 128, 32)
        .transpose(1, 0, 2)).reshape(128, 512).astype(np.uint16)
    m["ttm_bf"] = np.asarray(inputs["travel_time_matrix"], np.float32).astype(NBF)
    vis = np.asarray(inputs["visited"][sl]).astype(np.uint8)
    am = np.asarray(inputs["action_mask"][sl]).astype(np.uint8)
    visam = np.ascontiguousarray(np.concatenate([vis, am], axis=1))
    v = np.zeros((128, NG, 4, GB, 2), dtype=np.float32)
    v[:, :, :, :, 0] = 0.25
    v[:, :, :, :, 1] = 0.25 * vis.reshape(NG, GB, 4, 128).transpose(3, 0, 2, 1)
    vfc = np.ascontiguousarray(v).reshape(128, NG * 128).astype(NF8)
    wl = np.asarray(inputs["W_last"], np.float32)
    wf = np.asarray(inputs["W_first"], np.float32)
    wg = np.asarray(inputs["W_graph"], np.float32)
    wv = np.asarray(inputs["W_visited"], np.float32)
    wkT = np.asarray(inputs["W_key"], np.float32).T
    wcat = np.ascontiguousarray(
        np.concatenate([wl, wf, wg, wv, wkT], axis=1)).astype(NBF)
    wsp = np.zeros((128, 128), dtype=NBF)
    wsp[:3] = np.asarray(inputs["W_state"], np.float32).astype(NBF)
    bst = np.asarray(inputs["b_state"], np.float32).reshape(D, 1)
    sc8 = np.ascontiguousarray(np.concatenate(
        [np.asarray(inputs["current_time"][sl], np.float32),
         np.asarray(inputs["used_capacity"][sl], np.float32),
         np.asarray(inputs["vehicle_capacity"][sl], np.float32),
         np.asarray(inputs["i"][sl]).astype(np.float32),
         np.asarray(inputs["current_node"][sl]).astype(np.float32),
         np.asarray(inputs["previous_action"][sl]).astype(np.float32),
         np.asarray(inputs["first_node"][sl]).astype(np.float32).reshape(BC, 1),
         np.zeros((BC, 1), np.float32)], axis=1))
    c = consts or _consts()
    u8 = np.uint8
    m["blob"] = np.ascontiguousarray(np.concatenate([
        sc8.view(u8), c["iota"].view(u8), bst.view(u8), vfc.view(u8),
        visam, wcat.view(u8), wsp[:, :128].view(u8), c["ident"].view(u8),
        h3w.view(u8), c["sselc"].view(u8), c["dkc"].view(u8)], axis=1))
    assert m["blob"].shape == (128, BLOB_BYTES), m["blob"].shape
    return m


_last_results = None


def kernel(**inputs):
    global _last_results
    nc = _cached_program()
    consts = _consts()
    in_maps = [make_in_map(inputs, c, consts) for c in range(NCORES)]
    import os
    trace = bool(int(os.environ.get("KERNEL_TRACE", "0")))
    rr = run_bass_kernel_spmd(nc, in_maps, list(range(NCORES)), trace=trace)
    _last_results = rr
    out = np.concatenate([np.asarray(rr.results[c]["out"], np.float32)
                          for c in range(NCORES)], axis=0)
    return out


# revision 38
# speedup vs baseline: 1.2542x; 1.2542x over previous
"""Trainium2 Bass kernel for nn_DARPDecoder (sparse_attention).

Strategy (pure data-parallel over batch, 8 cores x 128 batches):
  score[b,n] = emb[b,n,:].qk[b]/sqrt(D) - travel[b,n]*c ; tanh-clip, mask,
  log_softmax, with qk[b] = W_key^T q[b] (no [B,N,D] K intermediate).
Per core, TWO HBM streams consumed in a chunk-wise pipeline:
  - nat_f8 (natural [n,d] tiles, fp8-e4m3, 8.4MB, 8 chunks of 16 batches):
    graph/visited sums via "flipped" matmuls -- the nat tile is the
    STATIONARY, the moving operand a tiny [128,2] (0.25 | 0.25*vf) column
    pair accumulating [128d,2] per batch.  fp8 is plenty for the sums.
  - et_bf (transposed [d,n] tiles, bf16, 16.8MB, 16 half-chunks): per-batch
    score matmuls with zero-padded qk columns (batch b -> col 32b+(b%32))
    accumulate into a [128b, 512n] PSUM, 32-row band per batch group.
Streams are ordered nat-first so every group's sums -> 1/vcount descale
(exact transpose sandwich) -> q -> qk completes while et streams in; scores
then chase each et half-chunk; the epilogue runs per 32-row band as its band
completes.  All consts ship as ONE u8 blob DMA (bitcast views); ~30 DMAs
total on one queue keeps the shared HWDGE/SEQ path off the critical path.
Travel: T rows gathered by cur_h3 (indirect DMA, bf16, pre-scaled by c via
16 selection matmuls that replicate rows across 16-partition groups), then
gpsimd indirect_copy gathers T[cur_h3[b], h3[b,n]], and 16 "select -1 rows"
matmuls accumulate -travel straight into the score PSUM (a zero-stationary
matmul provides the start=True clear, making accumulation order-free).
Epilogue per band: 10*tanh(x/10) = 10 - 20/(exp(x/5)+1) so tanh/exp/ln all
live in ONE activation-table set (a dummy Exp after each Ln re-pins it);
post-tanh scores are in [-10,10] so log-sum-exp needs no max shift.
"""

import functools
import math

import numpy as np
import ml_dtypes

import concourse.bass as bass
import concourse.mybir as mybir
import concourse.tile as tile
from concourse import bacc
from concourse.bass_utils import run_bass_kernel_spmd

BF16 = mybir.dt.bfloat16
F8 = mybir.dt.float8e4
F32 = mybir.dt.float32
I32 = mybir.dt.int32
U16 = mybir.dt.uint16
U8 = mybir.dt.uint8
Alu = mybir.AluOpType
AF = mybir.ActivationFunctionType
AX = mybir.AxisListType

B, N, D, NCORES = 1024, 512, 128, 8
BC = B // NCORES   # 128 batches/core
NG, GB = 8, 16     # 8 groups x 16 batches
MAX_TIME = 1440.0
TANH_CLIP = 10.0
C_TRAVEL = 1.0 / MAX_TIME / math.sqrt(2.0)
INV_SQRT_D = 1.0 / math.sqrt(D)
NBF = np.dtype(ml_dtypes.bfloat16)
NF8 = np.dtype(ml_dtypes.float8_e4m3)
BLOB_BYTES = 13096
DEBUG_TAPS = False


def _emit(nc, tc, T):
    ap = {k: v.ap() for k, v in T.items()}

    with (
        tc.tile_pool(name="cp", bufs=1) as cp,
        tc.tile_pool(name="wk", bufs=2) as wk,
        tc.tile_pool(name="stn", bufs=6) as stn,
        tc.tile_pool(name="ste", bufs=10) as ste,
        tc.tile_pool(name="ps_sc", bufs=1, space="PSUM") as ps_sc,
        tc.tile_pool(name="ps_rep", bufs=2, space="PSUM") as ps_rep,
        tc.tile_pool(name="ps_sm", bufs=2, space="PSUM") as ps_sm,
        tc.tile_pool(name="ps_q", bufs=2, space="PSUM") as ps_q,
    ):
        # ---- DMA issue: one consts blob, then the two streams, all on SP
        #      (one HWDGE slot per big transfer; never issue-rate bound) ----
        blob = cp.tile([128, BLOB_BYTES], U8, name="blob")
        nc.sync.dma_start(out=blob[:], in_=ap["blob"])
        sc8 = blob[:, 0:32].bitcast(F32)
        iota = blob[:, 32:36].bitcast(F32)
        bst = blob[:, 36:40].bitcast(F32)
        vfc = blob[:, 40:1064].bitcast(F8)
        visam = blob[:, 1064:2088]
        wcat = blob[:, 2088:3368].bitcast(BF16)
        ws = blob[:3, 3368:3624].bitcast(BF16)
        idn = blob[:, 3624:3880].bitcast(BF16)
        h3w = blob[:, 3880:4904].bitcast(U16)
        sselc = blob[:, 4904:9000].bitcast(BF16)
        dkc = blob[:, 9000:13096].bitcast(BF16)

        nat = [None] * NG
        for g in range(NG):
            nat[g] = stn.tile([128, GB * 4 * D], F8, tag="nat", name=f"nat{g}")
            nc.sync.dma_start(out=nat[g][:], in_=ap["nat_f8"][g])
        et = [None] * 16
        for h in range(16):
            et[h] = ste.tile([128, 8 * N], BF16, tag="et", name=f"et{h}")
            if h == 15:
                # split the final chunk so the last scores start sooner
                nc.sync.dma_start(out=et[h][:, :4 * N], in_=ap["et_bf"][h][:, :4 * N])
                nc.sync.dma_start(out=et[h][:, 4 * N:], in_=ap["et_bf"][h][:, 4 * N:])
            else:
                nc.sync.dma_start(out=et[h][:], in_=ap["et_bf"][h])

        # zero stationary for the pssc-clearing matmul
        zc = cp.tile([128, 128], BF16, name="zc")
        nc.vector.memset(zc[:], 0.0)

        # warm the ACT table set (copy/exp/ln all live in one set)
        actw = cp.tile([1, 1], F32, name="actw")
        nc.vector.memset(actw[:], 0.0)
        nc.scalar.activation(out=actw[:], in_=actw[:], func=AF.Exp)

        # ---- qkp (zero-padded qk columns), memset early: zero deps ----
        qkp = cp.tile([128, 32 * BC], BF16, name="qkp")
        nc.vector.memset(qkp[:], 0.0)

        # ---- scalar state / first-node bookkeeping (DVE, f32 exact) ----
        t1 = cp.tile([BC, 1], F32, name="t1")
        nc.vector.tensor_single_scalar(out=t1[:], in_=sc8[:, 5:6], scalar=0.0,
                                       op=Alu.is_equal)
        t2 = cp.tile([BC, 1], F32, name="t2")
        nc.vector.tensor_single_scalar(out=t2[:], in_=sc8[:, 4:5], scalar=0.0,
                                       op=Alu.not_equal)
        ld = cp.tile([BC, 1], F32, name="ld")
        nc.vector.tensor_mul(out=ld[:], in0=t1[:], in1=t2[:])
        dd = cp.tile([BC, 1], F32, name="dd")
        nc.vector.tensor_sub(out=dd[:], in0=sc8[:, 4:5], in1=sc8[:, 6:7])
        nc.vector.tensor_mul(out=dd[:], in0=ld[:], in1=dd[:])
        fnf = cp.tile([BC, 1], F32, name="fnf")
        nc.vector.tensor_add(out=fnf[:], in0=sc8[:, 6:7], in1=dd[:])
        nc.vector.tensor_mul(out=fnf[:], in0=fnf[:], in1=t2[:])

        gcf = cp.tile([BC, 1], F32, name="gcf")
        nc.vector.tensor_add(out=gcf[:], in0=iota, in1=sc8[:, 4:5])
        gcur = cp.tile([BC, 1], I32, name="gcur")
        nc.vector.tensor_copy(out=gcur[:], in_=gcf[:])
        gff = cp.tile([BC, 1], F32, name="gff")
        nc.vector.tensor_add(out=gff[:], in0=iota, in1=fnf[:])
        gfn = cp.tile([BC, 1], I32, name="gfn")
        nc.vector.tensor_copy(out=gfn[:], in_=gff[:])

        # visited count -> vcr4 = 4/max(vc,1) (bf16 col for ident matmul rhs)
        visf = cp.tile([BC, N], F32, name="visf")
        nc.vector.tensor_copy(out=visf[:], in_=visam[:, :N])
        vc = cp.tile([BC, 1], F32, name="vc")
        nc.vector.tensor_reduce(out=vc[:], in_=visf[:], axis=AX.X, op=Alu.add)
        nc.vector.tensor_scalar_max(out=vc[:], in0=vc[:], scalar1=1.0)
        nc.vector.tensor_scalar_mul(out=vc[:], in0=vc[:], scalar1=0.25)
        vcr4 = cp.tile([BC, 1], F32, name="vcr4")
        nc.vector.reciprocal(out=vcr4[:], in_=vc[:])
        vcr4b = cp.tile([BC, 1], BF16, name="vcr4b")
        nc.vector.tensor_copy(out=vcr4b[:], in_=vcr4[:])

        # action mask precompute
        amf = cp.tile([BC, N], F32, name="amf")
        nc.vector.tensor_copy(out=amf[:], in_=visam[:, N:])
        mA = cp.tile([BC, N], F32, name="mA")
        nc.vector.tensor_scalar(out=mA[:], in0=amf[:], scalar1=1.0, scalar2=1e8,
                                op0=Alu.subtract, op1=Alu.mult)
        m10 = cp.tile([BC, N], F32, name="m10")
        nc.vector.tensor_scalar_mul(out=m10[:], in0=amf[:], scalar1=TANH_CLIP)
        nc.vector.tensor_add(out=mA[:], in0=mA[:], in1=m10[:])
        mB = cp.tile([BC, N], F32, name="mB")
        nc.vector.tensor_scalar_mul(out=mB[:], in0=amf[:], scalar1=2.0 * TANH_CLIP)

        # state feats [BC,3] -> sft [3,BC]
        sfb = cp.tile([BC, 3], BF16, name="sfb")
        nc.vector.tensor_sub(out=sfb[:, 0:1], in0=sc8[:, 2:3], in1=sc8[:, 1:2])
        nc.vector.tensor_scalar_mul(out=sfb[:, 1:2], in0=sc8[:, 0:1],
                                    scalar1=1.0 / MAX_TIME)
        nc.vector.tensor_scalar_mul(out=sfb[:, 2:3], in0=sc8[:, 3:4],
                                    scalar1=1.0 / (2.0 * N))
        psf = ps_q.tile([128, 128], BF16, tag="sm")
        nc.tensor.transpose(out=psf[:3, :], in_=sfb[:], identity=idn)
        sft = cp.tile([3, BC], BF16, name="sft")
        nc.vector.tensor_copy(out=sft[:], in_=psf[:3, :BC])

        # wg scaled by 1/128 (sums use 0.25 weights; graph mean needs /512)
        wgs = cp.tile([D, D], BF16, name="wgs")
        nc.vector.tensor_scalar_mul(out=wgs[:], in0=wcat[:, 2 * D:3 * D],
                                    scalar1=1.0 / 128.0)

        # ---- gathers (Pool/SWDGE queue) ----
        hc_rows = cp.tile([BC, D], BF16, name="hc_rows")
        nc.gpsimd.indirect_dma_start(
            out=hc_rows, out_offset=None, in_=ap["emb_flat"],
            in_offset=bass.IndirectOffsetOnAxis(ap=gcur[:, :1], axis=0))
        hf_rows = cp.tile([BC, D], BF16, name="hf_rows")
        nc.gpsimd.indirect_dma_start(
            out=hf_rows, out_offset=None, in_=ap["emb_flat"],
            in_offset=bass.IndirectOffsetOnAxis(ap=gfn[:, :1], axis=0))
        ch3 = cp.tile([BC, 1], I32, name="ch3")
        nc.gpsimd.indirect_dma_start(
            out=ch3[:], out_offset=None, in_=ap["h3_flat"],
            in_offset=bass.IndirectOffsetOnAxis(ap=gcur[:, :1], axis=0))
        rrow = cp.tile([BC, N], BF16, name="rrow")
        nc.gpsimd.indirect_dma_start(
            out=rrow[:], out_offset=None, in_=ap["ttm_bf"],
            in_offset=bass.IndirectOffsetOnAxis(ap=ch3[:, :1], axis=0))

        # ---- h_cur / h_first transposes -> [128d, BC] bf16 ----
        hct = cp.tile([D, BC], BF16, name="hct")
        pt1 = ps_q.tile([128, 128], BF16, tag="sm")
        nc.tensor.transpose(out=pt1[:], in_=hc_rows, identity=idn)
        nc.vector.tensor_copy(out=hct[:], in_=pt1[:])
        hft = cp.tile([D, BC], BF16, name="hft")
        pt2 = ps_q.tile([128, 128], BF16, tag="sm")
        nc.tensor.transpose(out=pt2[:], in_=hf_rows, identity=idn)
        nc.vector.tensor_copy(out=hft[:], in_=pt2[:])

        gk_all = cp.tile([128, 16 * N], BF16, name="gk_all")
        pssc = ps_sc.tile([128, N], F32, tag="score")
        # clear pssc once; every later matmul (travel + scores) accumulates
        nc.tensor.matmul(out=pssc[:], lhsT=zc[:], rhs=qkp[:, :N], start=True,
                         stop=False, skip_group_check=True)

        # ---- loop A: per-group sums -> qk (chases the nat stream); travel
        #      replication/gather interleaved 2 calls per group ----
        for g in range(NG):
            # sums: flipped matmuls, nat tile slices as stationaries
            psS = ps_sm.tile([128, 2 * GB], F32, tag="sums")
            for j in range(GB):
                for t in range(4):
                    nc.tensor.matmul(
                        out=psS[:, 2 * j:2 * j + 2],
                        lhsT=nat[g][:, (j * 4 + t) * D:(j * 4 + t + 1) * D],
                        rhs=vfc[:, 128 * g + 32 * t + 2 * j:
                                 128 * g + 32 * t + 2 * j + 2],
                        start=(t == 0), stop=(t == 3), skip_group_check=True)

            # graph cols (even) / raw visited cols (odd) -> SBUF bf16
            gt_g = wk.tile([D, GB], BF16, tag="gt")
            nc.vector.tensor_copy(
                out=gt_g[:], in_=psS[:].rearrange("p (s c) -> p s c", c=2)[:, :, 0])
            vr_g = wk.tile([D, GB], BF16, tag="vr")
            nc.vector.tensor_copy(
                out=vr_g[:], in_=psS[:].rearrange("p (s c) -> p s c", c=2)[:, :, 1])

            # 1/vcount descale sandwich: transpose, per-partition scale, back
            vcg = ps_q.tile([GB, 1], F32, tag="sm")
            nc.tensor.matmul(out=vcg[:], lhsT=idn[:, GB * g:GB * (g + 1)],
                             rhs=vcr4b[:], start=True, stop=True)
            vcgs = wk.tile([GB, 1], F32, tag="vcgs")
            nc.vector.tensor_copy(out=vcgs[:], in_=vcg[:])
            pvt = ps_q.tile([GB, D], BF16, tag="sm")
            nc.tensor.transpose(out=pvt[:], in_=vr_g[:], identity=idn)
            vts = wk.tile([GB, D], BF16, tag="vts")
            nc.vector.tensor_scalar(out=vts[:], in0=pvt[:], scalar1=vcgs[:, :1],
                                    scalar2=None, op0=Alu.mult)
            pvb = ps_q.tile([D, GB], F32, tag="sm")
            nc.tensor.matmul(out=pvb[:], lhsT=vts[:], rhs=idn[:GB, :GB],
                             start=True, stop=True)
            vt_g = wk.tile([D, GB], BF16, tag="vt")
            nc.vector.tensor_copy(out=vt_g[:], in_=pvb[:])

            # q = W_last^T hc + W_first^T hf + Wg' G + Wv V + W_state^T sf (+b)
            psq = ps_q.tile([D, GB], F32, tag="sm")
            nc.tensor.matmul(out=psq[:], lhsT=wcat[:, 0:D],
                             rhs=hct[:, GB * g:GB * (g + 1)], start=True, stop=True)
            nc.tensor.matmul(out=psq[:], lhsT=wcat[:, D:2 * D],
                             rhs=hft[:, GB * g:GB * (g + 1)], start=False,
                             stop=True, skip_group_check=True)
            nc.tensor.matmul(out=psq[:], lhsT=wgs[:], rhs=gt_g[:], start=False,
                             stop=True, skip_group_check=True)
            nc.tensor.matmul(out=psq[:], lhsT=wcat[:, 3 * D:4 * D], rhs=vt_g[:],
                             start=False, stop=True, skip_group_check=True)
            nc.tensor.matmul(out=psq[:], lhsT=ws,
                             rhs=sft[:, GB * g:GB * (g + 1)], start=False,
                             stop=True, skip_group_check=True)
            qt_g = wk.tile([D, GB], BF16, tag="qt")
            nc.vector.tensor_scalar(out=qt_g[:], in0=psq[:], scalar1=bst[:, :1],
                                    scalar2=None, op0=Alu.add)

            # qk = W_key^T q / sqrt(D)
            psk = ps_q.tile([D, GB], F32, tag="sm")
            nc.tensor.matmul(out=psk[:], lhsT=wcat[:, 4 * D:5 * D], rhs=qt_g[:],
                             start=True, stop=True)
            qk_g = wk.tile([D, GB], BF16, tag="qkg")
            nc.vector.tensor_scalar_mul(out=qk_g[:], in0=psk[:],
                                        scalar1=INV_SQRT_D)

            # scatter into qkp: batch b=16g+j -> col 32b + (b%32)
            base = 512 * g + 16 * (g % 2)
            nc.vector.tensor_copy(out=qkp[:, base:base + 33 * (GB - 1) + 1:33],
                                  in_=qk_g[:])

            # travel replication + gpsimd gather, 2 calls per group
            for k in (2 * g, 2 * g + 1):
                prep = ps_rep.tile([128, N], F32, tag="rep")
                nc.tensor.matmul(out=prep[:], lhsT=sselc[:, 128 * k:128 * (k + 1)],
                                 rhs=rrow[:], start=True, stop=True)
                sck = wk.tile([128, N], BF16, tag="sck")
                nc.scalar.activation(out=sck[:], in_=prep[:], func=AF.Copy)
                nc.gpsimd.indirect_copy(out=gk_all[:, N * k:N * (k + 1)],
                                        data=sck[:],
                                        idxs=h3w[:, 32 * k:32 * (k + 1)],
                                        i_know_ap_gather_is_preferred=True)

        # ---- travel select-accumulate straight into the score PSUM ----
        for k in range(16):
            nc.tensor.matmul(out=pssc[:], lhsT=dkc[:, 128 * k:128 * (k + 1)],
                             rhs=gk_all[:, N * k:N * (k + 1)],
                             start=False, stop=False, skip_group_check=True)

        # ---- loop B: per-half-group scores (chases the et stream), with the
        #      epilogue + output DMA emitted per 32-row band as it completes ----
        th = cp.tile([BC, N], F32, name="th")
        msk = cp.tile([BC, N], F32, name="msk")
        ex = cp.tile([BC, N], F32, name="ex")
        fin = cp.tile([BC, N], F32, name="fin")
        se = cp.tile([BC, 2], F32, name="se")
        set_ = cp.tile([BC, 1], F32, name="set_")
        lse = cp.tile([BC, 1], F32, name="lse")
        for h in range(16):
            J = h // 4
            for j in range(8):
                b = 8 * h + j
                nc.tensor.matmul(
                    out=pssc[32 * J:32 * J + 32, :],
                    lhsT=qkp[:, 32 * b:32 * b + 32],
                    rhs=et[h][:, N * j:N * (j + 1)],
                    start=False, stop=(b % 32 == 31),
                    tile_position=(0, 32 * J), skip_group_check=True)
            if h % 4 == 3:
                sl = slice(32 * J, 32 * J + 32)
                # 10*tanh(x/10) = 10 - 20/(exp(x/5)+1): stays in the exp/ln
                # act-table set (no per-band table reloads).  Post-tanh scores
                # are clipped to [-10,10], so log-sum-exp needs no max shift.
                # The last band is column-split so the serial op chain
                # pipelines across ACT and DVE in the tail.
                halves = (slice(0, N),) if J < 3 else (slice(0, N // 2),
                                                       slice(N // 2, N))
                for ci, cs in enumerate(halves):
                    nc.scalar.activation(out=th[sl, cs], in_=pssc[sl, cs],
                                         func=AF.Exp, scale=2.0 / TANH_CLIP)
                    nc.vector.tensor_scalar_add(out=th[sl, cs], in0=th[sl, cs],
                                                scalar1=1.0)
                    nc.vector.reciprocal(out=th[sl, cs], in_=th[sl, cs])
                    nc.vector.tensor_mul(out=th[sl, cs], in0=th[sl, cs],
                                         in1=mB[sl, cs])
                    nc.vector.tensor_sub(out=msk[sl, cs], in0=mA[sl, cs],
                                         in1=th[sl, cs])
                    nc.scalar.activation(out=ex[sl, cs], in_=msk[sl, cs],
                                         func=AF.Exp, scale=1.0,
                                         accum_out=se[sl, ci:ci + 1])
                if J == 3:
                    nc.vector.tensor_add(out=set_[sl], in0=se[sl, 0:1],
                                         in1=se[sl, 1:2])
                    nc.scalar.activation(out=lse[sl], in_=set_[sl], func=AF.Ln)
                else:
                    nc.scalar.activation(out=lse[sl], in_=se[sl, 0:1],
                                         func=AF.Ln)
                nc.vector.tensor_scalar(out=fin[sl], in0=msk[sl],
                                        scalar1=lse[sl.start:sl.stop, :1],
                                        scalar2=None, op0=Alu.subtract)
                nc.sync.dma_start(out=ap["out"][sl], in_=fin[sl])
                if J < 3:
                    # re-pin the exp table so the next band's Exp needs no
                    # act-table reload (Ln lives in a different set)
                    nc.scalar.activation(out=actw[:], in_=actw[:], func=AF.Exp)



def build_program():
    nc = bacc.Bacc("TRN2", target_bir_lowering=False, debug=False)
    dt = nc.dram_tensor
    T = {}

    def din(name, shape, dtype):
        T[name] = dt(name, shape, dtype, kind="ExternalInput")

    din("nat_f8", [NG, 128, GB * 4 * D], F8)
    din("et_bf", [16, 128, 8 * N], BF16)
    din("emb_flat", [BC * N, D], BF16)
    din("h3_flat", [BC * N, 1], I32)
    din("ttm_bf", [N, N], BF16)
    din("blob", [128, BLOB_BYTES], U8)
    T["out"] = dt("out", [BC, N], F32, kind="ExternalOutput")
    if DEBUG_TAPS:
        for nm, shp in [("d_hct", [D, BC]), ("d_hft", [D, BC]),
                        ("d_qkp", [128, 32 * BC]), ("d_trav", [BC, N]),
                        ("d_score", [BC, N])]:
            T[nm] = dt(nm, shp, F32, kind="ExternalOutput")

    with tile.TileContext(nc) as tc:
        _emit(nc, tc, T)
    nc.compile()
    return nc


@functools.cache
def _cached_program():
    return build_program()


@functools.cache
def _consts():
    c = {}
    c["ident"] = np.eye(128, dtype=NBF)
    s = np.zeros((16, 128, 128), dtype=np.float32)
    dk = np.zeros((16, 128, 128), dtype=np.float32)
    pidx = np.arange(128)
    for k in range(16):
        s[k, (pidx // 16) * 16 + k, pidx] = C_TRAVEL
        rows = pidx[pidx % 16 == k]
        dk[k, rows, rows] = -1.0
    c["sselc"] = np.ascontiguousarray(s.transpose(1, 0, 2)).reshape(128, 2048).astype(NBF)
    c["dkc"] = np.ascontiguousarray(dk.transpose(1, 0, 2)).reshape(128, 2048).astype(NBF)
    c["iota"] = (np.arange(BC, dtype=np.float32) * N)[:, None]
    return c


def make_in_map(inputs, core, consts=None):
    """Host-side shard + relayout for one core (pure layout/dtype work)."""
    sl = slice(BC * core, BC * (core + 1))
    emb = np.asarray(inputs["node_emb"][sl], dtype=np.float32)
    embb = emb.astype(NBF)          # [128, 512, 128]
    embf8 = emb.astype(NF8)
    m = {}
    m["nat_f8"] = np.ascontiguousarray(
        embf8.reshape(NG, GB, 4, 128, D).transpose(0, 3, 1, 2, 4)
    ).reshape(NG, 128, GB * 4 * D)
    m["et_bf"] = np.ascontiguousarray(
        embb.transpose(0, 2, 1).reshape(16, 8, D, N).transpose(0, 2, 1, 3)
    ).reshape(16, 128, 8 * N)
    m["emb_flat"] = embb.reshape(BC * N, D)
    h3 = np.asarray(inputs["h3_indices"][sl]).astype(np.int32)   # [128, 512]
    m["h3_flat"] = h3.reshape(BC * N, 1)
    h3w = np.ascontiguousarray(
        h3.reshape(8, 16, 32, 16).transpose(1, 0, 3, 2).reshape(16, 128, 32)
        .transpose(1, 0, 2)).reshape(128, 512).astype(np.uint16)
    m["ttm_bf"] = np.asarray(inputs["travel_time_matrix"], np.float32).astype(NBF)
    vis = np.asarray(inputs["visited"][sl]).astype(np.uint8)
    am = np.asarray(inputs["action_mask"][sl]).astype(np.uint8)
    visam = np.ascontiguousarray(np.concatenate([vis, am], axis=1))
    v = np.zeros((128, NG, 4, GB, 2), dtype=np.float32)
    v[:, :, :, :, 0] = 0.25
    v[:, :, :, :, 1] = 0.25 * vis.reshape(NG, GB, 4, 128).transpose(3, 0, 2, 1)
    vfc = np.ascontiguousarray(v).reshape(128, NG * 128).astype(NF8)
    wl = np.asarray(inputs["W_last"], np.float32)
    wf = np.asarray(inputs["W_first"], np.float32)
    wg = np.asarray(inputs["W_graph"], np.float32)
    wv = np.asarray(inputs["W_visited"], np.float32)
    wkT = np.asarray(inputs["W_key"], np.float32).T
    wcat = np.ascontiguousarray(
        np.concatenate([wl, wf, wg, wv, wkT], axis=1)).astype(NBF)
    wsp = np.zeros((128, 128), dtype=NBF)
    wsp[:3] = np.asarray(inputs["W_state"], np.float32).astype(NBF)
    bst = np.asarray(inputs["b_state"], np.float32).reshape(D, 1)
    sc8 = np.ascontiguousarray(np.concatenate(
        [np.asarray(inputs["current_time"][sl], np.float32),
         np.asarray(inputs["used_capacity"][sl], np.float32),
         np.asarray(inputs["vehicle_capacity"][sl], np.float32),
         np.asarray(inputs["i"][sl]).astype(np.float32),
         np.asarray(inputs["current_node"][sl]).astype(np.float32),
         np.asarray(inputs["previous_action"][sl]).astype(np.float32),
         np.asarray(inputs["first_node"][sl]).astype(np.float32).reshape(BC, 1),
         np.zeros((BC, 1), np.float32)], axis=1))
    c = consts or _consts()
    u8 = np.uint8
    m["blob"] = np.ascontiguousarray(np.concatenate([
        sc8.view(u8), c["iota"].view(u8), bst.view(u8), vfc.view(u8),
        visam, wcat.view(u8), wsp[:, :128].view(u8), c["ident"].view(u8),
        h3w.view(u8), c["sselc"].view(u8), c["dkc"].view(u8)], axis=1))
    assert m["blob"].shape == (128, BLOB_BYTES), m["blob"].shape
    return m


_last_results = None


def kernel(**inputs):
    global _last_results
    nc = _cached_program()
    consts = _consts()
    in_maps = [make_in_map(inputs, c, consts) for c in range(NCORES)]
    import os
    trace = bool(int(os.environ.get("KERNEL_TRACE", "0")))
    rr = run_bass_kernel_spmd(nc, in_maps, list(range(NCORES)), trace=trace)
    _last_results = rr
    out = np.concatenate([np.asarray(rr.results[c]["out"], np.float32)
                          for c in range(NCORES)], axis=0)
    return out
